# revision 3
# baseline (speedup 1.0000x reference)
"""MoE (top-2 of 8 experts) Trainium2 kernel - F-split expert sharding,
resident weights.

Every core processes ALL routed (token, expert) pairs but only a 512-wide
slice of the hidden dim F: core c owns columns [c*512, (c+1)*512) of w1 and
the matching rows of w2. Per-core PE work is exactly total_work/8 regardless
of routing imbalance; the F-slice partial sums of layer 2 are combined on
the host (linear, so the result is unchanged).

All matmul operands are bf16 (PSUM accumulation stays fp32). w1/w2 for all
8 experts are DMA'd once into SBUF (128 KB/partition) and stay resident;
steady-state DMA is only the per-phase x slice (sync ring) and per-dc y
partials (scalar ring). x prefetch for phase i+1 is issued at the start of
phase i, giving the transfer a full phase (~27 us) of overlap window.

Per-expert phases: layer 1 tiles [128d x 128f] accumulate over 8 d-blocks
into PSUM, relu+bias evicted by ACT to bf16 h; layer 2 tiles [128f x 128d]
accumulate over 4 f-blocks, DVE-copied to bf16 y, DMA'd out per d-block.

Hardcoded problem shape: B=4, S=1024, D=1024, F=4096, E=8, TOP_K=2.
"""

import numpy as np

import concourse.bass as bass
import concourse.mybir as mybir
import concourse.tile as tile
from concourse import bacc
from concourse.bass_utils import run_bass_kernel_spmd

B, S, D, F, E = 4, 1024, 1024, 4096, 8
TOP_K = 2
P = 128
DC = D // P
FCL = F // E // P

_program_cache: dict = {}


def _split(n, max_piece=512):
    k = -(-n // max_piece)
    base = n // k
    pieces = [base] * k
    rem = n - base * k
    for i in range(rem):
        pieces[i] += 1
    out, acc = [], 0
    for p in pieces[:-1]:
        p4 = (p // 4) * 4
        out.append(p4)
        acc += p4
    out.append(n - acc)
    return out


def _build_program(pes, reps=1, x_bufs=2, h_bufs=2, y_bufs=10,
                   psh_bufs=4, psy_bufs=4, max_chunk=512):
    capt = sum(pes)
    pemax = max(pes)
    bf16 = mybir.dt.bfloat16
    f32 = mybir.dt.float32

    nc = bacc.Bacc("TRN2", target_bir_lowering=False, debug=False, num_devices=E)
    xT = nc.dram_tensor("xT", [DC, P, capt], bf16, kind="ExternalInput")
    w1p = nc.dram_tensor("w1p", [E, P, FCL * D], bf16, kind="ExternalInput")
    w2p = nc.dram_tensor("w2p", [E, P, DC * FCL * P], bf16, kind="ExternalInput")
    b1p = nc.dram_tensor("b1p", [P, E * FCL], f32, kind="ExternalInput")
    yT = nc.dram_tensor("yT", [DC, P, capt], bf16, kind="ExternalOutput")

    with tile.TileContext(nc) as tc:
        with (
            tc.tile_pool(name="consts", bufs=1) as consts,
            tc.tile_pool(name="xp", bufs=x_bufs) as xp,
            tc.tile_pool(name="hp", bufs=h_bufs) as hp,
            tc.tile_pool(name="yp", bufs=y_bufs) as yp,
            tc.tile_pool(name="psh", bufs=psh_bufs, space="PSUM") as psh,
            tc.tile_pool(name="psy", bufs=psy_bufs, space="PSUM") as psy,
        ):
            b1_sb = consts.tile([P, E * FCL], f32)
            nc.sync.dma_start(b1_sb[:], b1p[:])
            # resident weights: [P, E, FCL*D] each; loaded once, interleaved
            # per expert so phase 0's slices land first
            w1_sb = consts.tile([P, E, FCL * D], bf16)
            w2_sb = consts.tile([P, E, DC * FCL * P], bf16)

            offs = np.concatenate([[0], np.cumsum(pes)]).astype(int)
            phases = [(e, int(offs[e])) for _ in range(reps) for e in range(E)]

            def fetch_x(e):
                oe = int(offs[e])
                pe = pes[e]
                x_sb = xp.tile([P, DC, pemax], bf16, tag="x")
                nc.sync.dma_start(
                    x_sb[:, :, :pe],
                    xT[:, :, oe:oe + pe].rearrange("d p t -> p d t"))
                return x_sb

            # phase 0/1 weights first so compute unblocks early; weights for
            # e>=2 are queued progressively inside the early phases, behind
            # that phase's x prefetch
            nc.sync.dma_start(w1_sb[:, 0], w1p[0])
            nc.sync.dma_start(w2_sb[:, 0], w2p[0])
            fetched = fetch_x(phases[0][0])
            if E > 1:
                nc.sync.dma_start(w1_sb[:, 1], w1p[1])
                nc.sync.dma_start(w2_sb[:, 1], w2p[1])

            for i, (e, oe) in enumerate(phases):
                pe = pes[e]
                chunks = _split(pe, max_chunk)
                x_sb = fetched
                # one-phase x lookahead at phase start: with 2 buffers this
                # reuses x(i-1)'s buffer, whose readers finished last phase
                if i + 1 < len(phases):
                    fetched = fetch_x(phases[i + 1][0])
                if 2 + i < E:
                    nc.sync.dma_start(w1_sb[:, 2 + i], w1p[2 + i])
                    nc.sync.dma_start(w2_sb[:, 2 + i], w2p[2 + i])

                h_sb = hp.tile([P, FCL, pemax], bf16, tag="h")

                # layer 1: h[fc] = relu(sum_dc w1[fc,dc].T @ x[dc] + b1)
                for fc in range(FCL):
                    c0 = 0
                    for csz in chunks:
                        ph = psh.tile([P, max_chunk], f32, tag="ph")
                        for dc in range(DC):
                            nc.tensor.matmul(
                                ph[:, :csz],
                                w1_sb[:, e, (fc * DC + dc) * P:(fc * DC + dc + 1) * P],
                                x_sb[:, dc, c0:c0 + csz],
                                start=(dc == 0), stop=(dc == DC - 1),
                            )
                        nc.scalar.activation(
                            h_sb[:, fc, c0:c0 + csz], ph[:, :csz],
                            mybir.ActivationFunctionType.Relu,
                            bias=b1_sb[:, e * FCL + fc:e * FCL + fc + 1],
                        )
                        c0 += csz

                # layer 2: per dc, accumulate over fc; DMA the dc slice of y
                # out as soon as its copies are done
                for dc in range(DC):
                    y_sb = yp.tile([P, pemax], bf16, tag="y")
                    c0 = 0
                    for csz in chunks:
                        py = psy.tile([P, max_chunk], f32, tag="py")
                        for fc in range(FCL):
                            nc.tensor.matmul(
                                py[:, :csz],
                                w2_sb[:, e, (dc * FCL + fc) * P:(dc * FCL + fc + 1) * P],
                                h_sb[:, fc, c0:c0 + csz],
                                start=(fc == 0), stop=(fc == FCL - 1),
                            )
                        nc.vector.tensor_copy(
                            y_sb[:, c0:c0 + csz], py[:, :csz])
                        c0 += csz
                    nc.scalar.dma_start(
                        yT[dc, :, oe:oe + pe], y_sb[:, :pe])
    nc.finalize()
    return nc


def _route(x2d, gate_w, gate_b):
    logits = (x2d @ gate_w + gate_b).astype(np.float64)
    logits -= logits.max(axis=-1, keepdims=True)
    p = np.exp(logits)
    p /= p.sum(axis=-1, keepdims=True)
    order = np.argsort(-p, axis=-1)[:, :TOP_K]
    idx = []
    cw = []
    for e in range(E):
        sel = np.nonzero((order == e).any(axis=-1))[0]
        idx.append(sel)
        cw.append(p[sel, e].astype(np.float32))
    return idx, cw


def _pad4(n):
    return max(4, -(-n // 4) * 4)


def _pack_inputs(x2d, idx, w1, b1, w2):
    import ml_dtypes
    bf16 = ml_dtypes.bfloat16
    pes = [_pad4(len(i)) for i in idx]
    capt = sum(pes)

    xcat = np.zeros((capt, D), np.float32)
    oe = 0
    for e in range(E):
        xcat[oe:oe + len(idx[e])] = x2d[idx[e]]
        oe += pes[e]
    xT = np.ascontiguousarray(xcat.T.reshape(DC, P, capt).astype(bf16))

    in_maps = []
    for c in range(E):
        lo, hi = c * FCL * P, (c + 1) * FCL * P
        w1p = np.stack([
            np.ascontiguousarray(
                w1[e][:, lo:hi].reshape(DC, P, FCL, P)
                .transpose(1, 2, 0, 3).reshape(P, FCL * D))
            for e in range(E)])
        w2p = np.stack([
            np.ascontiguousarray(
                w2[e][lo:hi, :].reshape(FCL, P, DC, P)
                .transpose(1, 2, 0, 3).reshape(P, DC * FCL * P))
            for e in range(E)])
        b1p = np.stack([
            b1[e][lo:hi].reshape(FCL, P).T for e in range(E)],
            axis=1).reshape(P, E * FCL)
        in_maps.append({
            "xT": xT,
            "w1p": np.ascontiguousarray(w1p.astype(bf16)),
            "w2p": np.ascontiguousarray(w2p.astype(bf16)),
            "b1p": np.ascontiguousarray(b1p.astype(np.float32)),
        })
    return in_maps, pes


def kernel(x, gate_w, gate_b, w1, b1, w2, b2, _run_kwargs=None, _out=None):
    x = np.asarray(x, np.float32)
    gate_w = np.asarray(gate_w, np.float32)
    gate_b = np.asarray(gate_b, np.float32)
    w1 = np.asarray(w1, np.float32)
    b1 = np.asarray(b1, np.float32)
    w2 = np.asarray(w2, np.float32)
    b2 = np.asarray(b2, np.float32)

    x2d = x.reshape(-1, D)
    idx, cw = _route(x2d, gate_w, gate_b)
    in_maps, pes = _pack_inputs(x2d, idx, w1, b1, w2)
    key = tuple(pes)
    if key not in _program_cache:
        _program_cache[key] = _build_program(pes)
    nc = _program_cache[key]

    res = None
    for attempt in range(3):
        try:
            res = run_bass_kernel_spmd(
                nc, in_maps, core_ids=list(range(E)), **(_run_kwargs or {})
            )
            break
        except Exception:
            # transient device states (e.g. NRT_EXEC_UNIT_UNRECOVERABLE)
            # usually clear on retry; a wedged PJRT client needs a backend
            # reset first
            if attempt == 2:
                raise
            import time as _time
            _time.sleep(15)
            try:
                import jax
                jax.clear_caches()
                jax.extend.backend.clear_backends()
            except Exception:
                pass
    if _out is not None:
        _out.append(res)

    capt = sum(pes)
    ysum = np.zeros((capt, D), np.float32)
    for c in range(E):
        ysum += res.results[c]["yT"].astype(np.float32) \
            .transpose(2, 0, 1).reshape(capt, D)

    out = np.zeros((B * S, D), np.float32)
    oe = 0
    for e in range(E):
        n_e = len(idx[e])
        out[idx[e]] += cw[e][:, None] * (ysum[oe:oe + n_e] + b2[e])
        oe += pes[e]
    return out.reshape(B, S, D)
